# revision 22
# baseline (speedup 1.0000x reference)
"""Contextual patches score kernel for Trainium2 (8 NeuronCores).

Computes, per sample i:
    fs = f[i, :, ::2, ::2]; bs = b[i, :, ::2, ::2]          # [64, 80, 80]
    w  = 3x3 patches of bs (SAME, stride 1)                  # [6400, 64, 3, 3]
    wn = w / max(||w||_2, 1e-4)
    y[i] = conv(fs, wn, SAME)                                # [6400, 80, 80]

Implementation: y[l, p] = (w_l . f_patch_p) * inv_norm_l is a
[6400, 576] x [576, 6400] matmul per sample.  Sharding: 8 cores =
2 samples x 4 spatial-row quarters; each core computes [6400, 1600].
K = 576 = 64 channels x 9 taps, packed as 5 chunks of 128 partitions
(tap pairs stacked; last chunk zero-padded).  Operands are built once
in SBUF by copying shifted windows out of zero-padded images (a
row-shifted replica of each image lives in partitions 64-127 so a tap
pair is a single lane-aligned copy); fine-grained operand tiles let
the matmul stream start after ~2 tiles of build.  float32r matmuls
(full-rate at moving-dim >= 256, ~1e-4 rel err).  Patch normalization
is applied as a per-output-row scale after PSUM accumulation.
Patch norms: ACT squares the weight chunks, DVE sums them in F32
(f32r-input DVE arithmetic is silently wrong on this HW; the final add
writes an F32R tile), and a single ones-matmul per m-tile does the
partition reduction; the first 8 m-tiles use 5 ones-matmuls instead so
the DVE adds stay out of the build-congested startup ramp.
Measured ~230us/core on TRN2 (rel err 1.45e-4; PE-bound at ~199us).
"""

import ml_dtypes
import numpy as np

import concourse.bass as bass
import concourse.mybir as mybir
import concourse.tile as tile
from concourse.bass_utils import run_bass_kernel_spmd

F32 = mybir.dt.float32
F32R = mybir.dt.float32r
BF16 = mybir.dt.bfloat16
AF = mybir.ActivationFunctionType
NP_BF16 = ml_dtypes.bfloat16

C = 64            # channels
H = W = 80        # downsampled spatial size
L = H * W         # 6400 patches per sample
QROWS = 20        # output rows handled per core
POS = QROWS * W   # 1600 output positions per core
NTILE = 400       # matmul moving free dim (5 rows x 80)
NT = POS // NTILE         # 4 n-tiles
MT = L // 128             # 50 m-tiles
HALF_MT = MT // 2         # 25 (lhsT is split in two halves for pipelining)
NCHUNK = 5                # K chunks: 4 full tap pairs + 1 half (tap 8)
EPS = 1e-4

# chunk -> ((kh, kw) for partitions 0:64, (kh, kw) for partitions 64:128)
# The replica half of each padded image is shifted up one row, so a
# (kh, kw) / (kh+1, kw') pair reads with a single AP offset per half.
_CHUNK_TAPS = [
    ((0, 0), (1, 0)),
    ((0, 1), (1, 1)),
    ((0, 2), (1, 2)),
    ((2, 0), (2, 1)),
    ((2, 2), None),
]


def _win(img, kh, kw, nrows):
    """[*, nrows, 80] shifted window of a padded [*, rows, 82] image tile."""
    return img[:, kh:kh + nrows, kw:kw + W]


_COPY_SEQ = [0]


def _copy_chunk(nc, dst3, img, nrows, j):
    """Fill chunk j of dst3 [128, 5, nrows*80] with im2col windows.

    img: [128, nrows+2, 82] padded image; partitions 64:128 hold the
    same image shifted up one row (img2[c, r, x] = img1[c, r+1, x]).
    Chunk 4 holds tap (2,2) in BOTH halves (lower via base image, upper
    via replica) so K=64 chunk-4 matmuls can pair via tile_position.
    """
    def dst(j, p0, p1):
        return dst3[p0:p1, j, :].rearrange("p (y x) -> p y x", x=W)

    def copy(out, in_):
        # DVE:ACT = 2:1 (ACT copies are slower; this balances the two
        # engines).  The first 8 copies (which gate the first matmuls)
        # stay DVE-only: ACT starts ~3.5us late (cold boot).
        i = _COPY_SEQ[0]
        _COPY_SEQ[0] += 1
        if i < 8 or i % 3 != 2:
            nc.vector.tensor_copy(out, in_)
        else:
            nc.scalar.activation(out, in_, AF.Copy)

    if j < 3:
        (kh, kw), _ = _CHUNK_TAPS[j]
        copy(dst(j, 0, 128), _win(img, kh, kw, nrows))
    elif j == 3:
        # tap (2,0) from base half, tap (2,1) via replica (kh-1 index)
        copy(dst(3, 0, 64), _win(img[0:64], 2, 0, nrows))
        copy(dst(3, 64, 128), _win(img[64:128], 1, 1, nrows))
    else:
        # tap (2,2) in both halves (upper via replica at (1,2))
        copy(dst(4, 0, 64), _win(img[0:64], 2, 2, nrows))
        copy(dst(4, 64, 128), _win(img[64:128], 1, 2, nrows))


def build_nc():
    _COPY_SEQ[0] = 0
    nc = bass.Bass(target_bir_lowering=False)
    fs_d = nc.dram_tensor("fs_pad", [C, QROWS + 2, 82], BF16, kind="ExternalInput")
    bs_d = nc.dram_tensor("bs_pad", [C, 82, 82], BF16, kind="ExternalInput")
    y_d = nc.dram_tensor("y", [L, POS], BF16, kind="ExternalOutput")

    with tile.TileContext(nc) as tc:
        with (
            tc.tile_pool(name="big", bufs=1) as big,
            tc.tile_pool(name="pad", bufs=2) as padp,
            tc.tile_pool(name="sq", bufs=2) as sqp,
            tc.tile_pool(name="inv", bufs=4) as invp,
            tc.tile_pool(name="outp", bufs=3) as outp,
            tc.tile_pool(name="ps", bufs=6, space="PSUM") as psp,
            tc.tile_pool(name="pss", bufs=1, space="PSUM") as pssp,
        ):
            ones = big.tile([128, 2], BF16, tag="ones")
            nc.vector.memset(ones[:], 1.0)

            # f image quarter + row-shifted replica in partitions 64:128
            fpad = big.tile([128, QROWS + 2, 82], BF16, tag="fpad")
            nc.sync.dma_start(fpad[0:64, 0:11], fs_d[:, 0:11])
            nc.sync.dma_start(fpad[0:64, 11:QROWS + 2], fs_d[:, 11:QROWS + 2])
            nc.sync.dma_start(fpad[64:128, 0:11], fs_d[:, 1:12])
            nc.sync.dma_start(fpad[64:128, 11:QROWS + 1], fs_d[:, 12:QROWS + 2])

            # rhs: im2col of the f quarter, one [128, 5, 800] tile per
            # n-tile pair.  lhsT: b patches (transposed weights) in
            # [128, 5, 640] tiles (lcm(80,128): 8 image rows = exactly 5
            # m-tiles each).  The first rhs/lhsT tiles are built chunk-
            # interleaved so the first matmuls gate on ~2 copies; the
            # rest of the build overlaps the matmul stream.
            rhs = [big.tile([128, NCHUNK, POS // 2], BF16, tag=f"rhs{u}",
                            name=f"rhs{u}") for u in range(2)]
            lhsT = [big.tile([128, NCHUNK, 640], BF16, tag=f"lhsT{t}",
                             name=f"lhsT{t}") for t in range(MT // 5)]

            def build_rhs(u, j):
                _copy_chunk(nc, rhs[u], fpad[:, 10 * u:10 * u + 12, :],
                            QROWS // 2, j)

            def dma_bt(t):
                bt = padp.tile([128, 10, 82], BF16, tag="bpad")
                nc.sync.dma_start(bt[0:64], bs_d[:, 8 * t:8 * t + 10])
                nc.sync.dma_start(
                    bt[64:128, 0:9], bs_d[:, 8 * t + 1:8 * t + 10]
                )
                return bt

            # dummy warmup matmuls interleaved with the build: keep the
            # PE's HAM activity window busy so the real matmul stream
            # starts at the warm 2.4GHz clock instead of ramping at 1.2
            ps_warm = pssp.tile([128, 2], F32, tag="warm")
            warmed = [0]

            def warm_mm(dep_ap):
                if warmed[0] >= 12:
                    return
                warmed[0] += 1
                nc.tensor.matmul(
                    ps_warm[0:2, :], lhsT=dep_ap, rhs=ones[:],
                    start=True, stop=True,
                )

            bt0 = dma_bt(0)
            for j in range(NCHUNK):
                build_rhs(0, j)
                _copy_chunk(nc, lhsT[0], bt0, 8, j)
                build_rhs(1, j)
                warm_mm(lhsT[0][:, j, 0:2])
                warm_mm(rhs[1][:, j, 0:2])

            def build_tile(t):
                bt = dma_bt(t)
                for j in range(NCHUNK):
                    _copy_chunk(nc, lhsT[t], bt, 8, j)

            def norm_tile(t):
                # inv_norms for a whole lhsT tile (5 m-tiles, 640 patches)
                # in one batched chain: one bf16 Square on ACT, 4 bf16 DVE
                # adds, 5 small bf16 ones-matmuls (128-partition reduce)
                # into one PSUM tile, one Sqrt/max/reciprocal epilogue.
                sq = sqp.tile([128, NCHUNK, 640], BF16, tag="sq")
                nc.scalar.activation(sq[:], lhsT[t][:], AF.Square)
                # chunk 4's upper half duplicates tap (2,2) (for matmul
                # pairing) -- include it only on partitions 0:64
                ssum = sqp.tile([128, 640], BF16, tag="ssum")
                nc.vector.tensor_add(
                    ssum[0:64, :], sq[0:64, 0, :], sq[0:64, 4, :]
                )
                nc.vector.tensor_copy(ssum[64:128, :], sq[64:128, 0, :])
                nc.vector.tensor_add(ssum[:], ssum[:], sq[:, 1, :])
                nc.vector.tensor_add(ssum[:], ssum[:], sq[:, 2, :])
                ssr = sqp.tile([128, 640], BF16, tag="ssr")
                nc.vector.tensor_add(ssr[:], ssum[:], sq[:, 3, :])
                ps_w = pssp.tile([128, 6], F32, tag="pss")
                for ml in range(5):
                    nc.tensor.matmul(
                        ps_w[:, ml:ml + 2],
                        lhsT=ssr[:, ml * 128:(ml + 1) * 128],
                        rhs=ones[:],
                        start=True, stop=True,
                    )
                inv = invp.tile([128, 5], F32, tag="inv")
                nc.scalar.activation(inv[:], ps_w[:, 0:5], AF.Sqrt)
                nc.vector.tensor_scalar(
                    inv[:], inv[:], EPS, None, mybir.AluOpType.max
                )
                nc.vector.reciprocal(inv[:], inv[:])
                return inv

            # norms for the first two lhsT tiles issue right after their
            # builds (ahead of the remaining builds), so the m=0..9
            # scale-copies don't stall behind the build queue on ACT
            inv_of = {0: norm_tile(0)}
            build_tile(1)
            inv_of[1] = norm_tile(1)
            for t in range(2, MT // 5):
                build_tile(t)

            inv_t = None
            for m in range(MT):
                t, ml = divmod(m, 5)
                msl = slice(ml * 128, (ml + 1) * 128)
                tail_dma = m >= MT - 1

                if ml == 0:
                    inv_t = inv_of.get(t)
                    if inv_t is None:
                        inv_t = norm_tile(t)
                inv = inv_t[:, ml:ml + 1]
                # chunk 4 (tap (2,2), K=64) leads each accumulation pair
                # via tile_position row groups -- nt 0/2 read the lower
                # half, nt 1/3 the (replica-filled) upper half.  The two
                # K=64 matmuls run concurrently (disjoint row groups) and
                # their weight loads hide under each other; leading the
                # group keeps their LDW off the full-array matmuls' rows.
                pstiles = []
                for nt in range(NT):
                    ps = psp.tile([128, NTILE], F32, tag="ps")
                    pstiles.append(ps)
                    p0 = 64 * (nt % 2)
                    nsl = slice((nt % 2) * NTILE, (nt % 2 + 1) * NTILE)
                    nc.tensor.matmul(
                        ps[:],
                        lhsT=lhsT[t][p0:p0 + 64, 4, msl],
                        rhs=rhs[nt // 2][p0:p0 + 64, 4, nsl],
                        start=True,
                        stop=False,
                        tile_position=(p0, 0),
                    )
                    if nt % 2 == 1:
                        for jnt in (nt - 1, nt):
                            jsl = slice((jnt % 2) * NTILE,
                                        (jnt % 2 + 1) * NTILE)
                            for j in range(NCHUNK - 1):
                                nc.tensor.matmul(
                                    pstiles[jnt][:],
                                    lhsT=lhsT[t][:, j, msl],
                                    rhs=rhs[jnt // 2][:, j, jsl],
                                    start=False,
                                    stop=(j == NCHUNK - 2),
                                )

                # n-tiles in pairs sharing one [128, 800] output staging
                # tile -> one DMA per pair (halves Sync-sequencer issues).
                # The last m-tile instead issues 4 half-partition DMAs per
                # pair: a single [128, 800] DMA occupies one HW queue for
                # ~10us, which would otherwise be the kernel tail.
                for nt0 in range(0, NT, 2):
                    ot = outp.tile([128, 2, NTILE], BF16, tag="ot")
                    for i, nt in enumerate((nt0, nt0 + 1)):
                        # alternate DVE / ACT: after the norm rework both
                        # PE and DVE sit at ~198us while ACT has ~125us
                        # slack; splitting the 200 scale-copies rebalances
                        # DVE and ACT to ~160us each
                        if i == 0:
                            nc.vector.tensor_scalar_mul(
                                ot[:, i, :], pstiles[nt][:], inv
                            )
                        else:
                            nc.scalar.activation(
                                ot[:, i, :], pstiles[nt][:], AF.Copy,
                                scale=inv,
                            )
                        if tail_dma:
                            for p0 in (0, 64):
                                nc.sync.dma_start(
                                    y_d[m * 128 + p0:m * 128 + p0 + 64,
                                        nt * NTILE:(nt + 1) * NTILE],
                                    ot[p0:p0 + 64, i, :],
                                )
                    if not tail_dma:
                        nc.sync.dma_start(
                            y_d[m * 128:(m + 1) * 128,
                                nt0 * NTILE:(nt0 + 2) * NTILE],
                            ot[:],
                        )
    return nc


def _split_multiwaits(nc, maxw=1):
    """Walrus (this build) accepts at most one sync-wait per instruction.

    Tile's kernel-tail drain carries one wait per active logical proc, so
    hoist excess waits onto same-engine NoOps inserted right before the
    offending instruction (engine executes them in order -> identical
    blocking semantics)."""
    n = 0
    for fn in nc.m.functions:
        for blk in fn.blocks:
            insts = list(blk.instructions)
            new, changed = [], False
            for ins in insts:
                si = ins.sync_info
                if si is not None and len(si.on_wait) > maxw:
                    extra, keep = si.on_wait[:-maxw], si.on_wait[-maxw:]
                    k = 0
                    while extra:
                        chunk, extra = extra[:maxw], extra[maxw:]
                        new.append(mybir.InstNoOp(
                            name=f"{ins.name}-ws{k}",
                            engine=ins.engine,
                            bass_nofuse=True,
                            sync_info=mybir.SyncInfo(
                                on_wait=list(chunk), on_update=[]
                            ),
                        ))
                        k += 1
                        n += 1
                    ins.sync_info = mybir.SyncInfo(
                        on_wait=list(keep), on_update=list(si.on_update)
                    )
                    changed = True
                new.append(ins)
            if changed:
                blk.instructions = new
    return n


_CACHE = {}


def _get_nc():
    if "nc" not in _CACHE:
        nc = build_nc()
        _split_multiwaits(nc)
        _CACHE["nc"] = nc
    return _CACHE["nc"]


def make_in_maps(f, b):
    f = np.asarray(f, dtype=np.float32)
    b = np.asarray(b, dtype=np.float32)
    n_samples = f.shape[0]
    fs = f[:, :, ::2, ::2].astype(NP_BF16)
    bs = b[:, :, ::2, ::2].astype(NP_BF16)
    fpad = np.zeros((n_samples, C, 82, 82), NP_BF16)
    fpad[:, :, 1:81, 1:81] = fs
    bpad = np.zeros((n_samples, C, 82, 82), NP_BF16)
    bpad[:, :, 1:81, 1:81] = bs
    in_maps = []
    for c in range(8):
        n, q = divmod(c, 4)
        in_maps.append({
            "fs_pad": np.ascontiguousarray(fpad[n, :, 20 * q:20 * q + 22, :]),
            "bs_pad": np.ascontiguousarray(bpad[n]),
        })
    return in_maps


def assemble(results, n_samples=2):
    out = np.empty((n_samples, L, H, W), np.float32)
    for c in range(8):
        n, q = divmod(c, 4)
        out[n, :, 20 * q:20 * q + 20, :] = (
            results[c]["y"].astype(np.float32).reshape(L, QROWS, W)
        )
    return out


def run(f, b, **kw):
    res = run_bass_kernel_spmd(_get_nc(), make_in_maps(f, b), list(range(8)), **kw)
    return assemble(res.results, np.asarray(f).shape[0]), res


def kernel(f, b):
    out, _ = run(f, b)
    return out



# revision 28
# speedup vs baseline: 1.0362x; 1.0362x over previous
"""Contextual patches score kernel for Trainium2 (8 NeuronCores).

Computes, per sample i:
    fs = f[i, :, ::2, ::2]; bs = b[i, :, ::2, ::2]          # [64, 80, 80]
    w  = 3x3 patches of bs (SAME, stride 1)                  # [6400, 64, 3, 3]
    wn = w / max(||w||_2, 1e-4)
    y[i] = conv(fs, wn, SAME)                                # [6400, 80, 80]

Implementation: y[l, p] = (w_l . f_patch_p) * inv_norm_l is a
[6400, 576] x [576, 6400] matmul per sample.  Sharding: 8 cores =
2 samples x 4 spatial-row quarters; each core computes [6400, 1600].
K = 576 = 64 channels x 9 taps, packed as 4 full chunks of 128
partitions (tap pairs stacked via a row-shifted image replica in
partitions 64-127) plus a half chunk (tap (2,2), K=64) that is
duplicated into both partition halves so two n-tiles' chunk-4 matmuls
run CONCURRENTLY via tile_position row groups (18 PE slots per m-tile
instead of 20; rel-err budget is 2e-2 so everything runs in bf16:
images are cast on the host, matmuls are bf16/FWL, the output is
DMA'd as bf16 and upcast on the host).  Patch normalization is a
per-output-row scale applied during the PSUM->SBUF drain (DVE/ACT
alternating).  Norms: per lhsT tile, one batched bf16 Square + 4 DVE
adds (excluding the duplicated tap), 5 tiny ones-matmuls for the
partition reduce, one Sqrt/max/reciprocal epilogue; tile 0 uses 5
single-m-tile chains instead so inv_0 is ready before m=0 drains.
Input/tail DMA triggers split across the two HWDGE rings (Sync+ACT).
Measured ~193us/core on TRN2 (rel err 3.9e-3; PE busy ~163us =
800 full matmul slots @167ns + chunk-4 pairs + LDW-transition stalls).
"""

import ml_dtypes
import numpy as np

import concourse.bass as bass
import concourse.mybir as mybir
import concourse.tile as tile
from concourse.bass_utils import run_bass_kernel_spmd

F32 = mybir.dt.float32
F32R = mybir.dt.float32r
BF16 = mybir.dt.bfloat16
AF = mybir.ActivationFunctionType
NP_BF16 = ml_dtypes.bfloat16

C = 64            # channels
H = W = 80        # downsampled spatial size
L = H * W         # 6400 patches per sample
QROWS = 20        # output rows handled per core
POS = QROWS * W   # 1600 output positions per core
NTILE = 400       # matmul moving free dim (5 rows x 80)
NT = POS // NTILE         # 4 n-tiles
MT = L // 128             # 50 m-tiles
HALF_MT = MT // 2         # 25 (lhsT is split in two halves for pipelining)
NCHUNK = 5                # K chunks: 4 full tap pairs + 1 half (tap 8)
EPS = 1e-4

# chunk -> ((kh, kw) for partitions 0:64, (kh, kw) for partitions 64:128)
# The replica half of each padded image is shifted up one row, so a
# (kh, kw) / (kh+1, kw') pair reads with a single AP offset per half.
_CHUNK_TAPS = [
    ((0, 0), (1, 0)),
    ((0, 1), (1, 1)),
    ((0, 2), (1, 2)),
    ((2, 0), (2, 1)),
    ((2, 2), None),
]


def _win(img, kh, kw, nrows):
    """[*, nrows, 80] shifted window of a padded [*, rows, 82] image tile."""
    return img[:, kh:kh + nrows, kw:kw + W]


_COPY_SEQ = [0]


def _copy_chunk(nc, dst3, img, nrows, j):
    """Fill chunk j of dst3 [128, 5, nrows*80] with im2col windows.

    img: [128, nrows+2, 82] padded image; partitions 64:128 hold the
    same image shifted up one row (img2[c, r, x] = img1[c, r+1, x]).
    Chunk 4 holds tap (2,2) in BOTH halves (lower via base image, upper
    via replica) so K=64 chunk-4 matmuls can pair via tile_position.
    """
    def dst(j, p0, p1):
        return dst3[p0:p1, j, :].rearrange("p (y x) -> p y x", x=W)

    def copy(out, in_):
        # DVE:ACT = 2:1 (ACT copies are slower; this balances the two
        # engines).  The first 8 copies (which gate the first matmuls)
        # stay DVE-only: ACT starts ~3.5us late (cold boot).
        i = _COPY_SEQ[0]
        _COPY_SEQ[0] += 1
        if i < 8 or i % 3 != 2:
            nc.vector.tensor_copy(out, in_)
        else:
            nc.scalar.activation(out, in_, AF.Copy)

    if j < 3:
        (kh, kw), _ = _CHUNK_TAPS[j]
        copy(dst(j, 0, 128), _win(img, kh, kw, nrows))
    elif j == 3:
        # tap (2,0) from base half, tap (2,1) via replica (kh-1 index)
        copy(dst(3, 0, 64), _win(img[0:64], 2, 0, nrows))
        copy(dst(3, 64, 128), _win(img[64:128], 1, 1, nrows))
    else:
        # tap (2,2) in both halves (upper via replica at (1,2))
        copy(dst(4, 0, 64), _win(img[0:64], 2, 2, nrows))
        copy(dst(4, 64, 128), _win(img[64:128], 1, 2, nrows))


def build_nc():
    _COPY_SEQ[0] = 0
    nc = bass.Bass(target_bir_lowering=False)
    fs_d = nc.dram_tensor("fs_pad", [C, QROWS + 2, 82], BF16, kind="ExternalInput")
    bs_d = nc.dram_tensor("bs_pad", [C, 82, 82], BF16, kind="ExternalInput")
    y_d = nc.dram_tensor("y", [L, POS], BF16, kind="ExternalOutput")

    with tile.TileContext(nc) as tc:
        with (
            tc.tile_pool(name="big", bufs=1) as big,
            tc.tile_pool(name="pad", bufs=2) as padp,
            tc.tile_pool(name="sq", bufs=2) as sqp,
            tc.tile_pool(name="inv", bufs=4) as invp,
            tc.tile_pool(name="outp", bufs=3) as outp,
            tc.tile_pool(name="ps", bufs=7, space="PSUM") as psp,
            tc.tile_pool(name="pss", bufs=1, space="PSUM") as pssp,
        ):
            ones = big.tile([128, 2], BF16, tag="ones")
            nc.vector.memset(ones[:], 1.0)

            # f image quarter + row-shifted replica in partitions 64:128.
            # DMA triggers cost ~0.6us each on their sequencer; split them
            # across the two HWDGE rings (Sync + Scalar) to halve the
            # serial input-DMA latency at startup.
            fpad = big.tile([128, QROWS + 2, 82], BF16, tag="fpad")
            nc.sync.dma_start(fpad[0:64], fs_d[:])
            nc.scalar.dma_start(
                fpad[64:128, 0:QROWS + 1], fs_d[:, 1:QROWS + 2]
            )

            # rhs: im2col of the f quarter, one [128, 5, 800] tile per
            # n-tile pair.  lhsT: b patches (transposed weights) in
            # [128, 5, 640] tiles (lcm(80,128): 8 image rows = exactly 5
            # m-tiles each).  The first rhs/lhsT tiles are built chunk-
            # interleaved so the first matmuls gate on ~2 copies; the
            # rest of the build overlaps the matmul stream.
            rhs = [big.tile([128, NCHUNK, POS // 2], BF16, tag=f"rhs{u}",
                            name=f"rhs{u}") for u in range(2)]
            lhsT = [big.tile([128, NCHUNK, 640], BF16, tag=f"lhsT{t}",
                             name=f"lhsT{t}") for t in range(MT // 5)]

            def build_rhs(u, j):
                _copy_chunk(nc, rhs[u], fpad[:, 10 * u:10 * u + 12, :],
                            QROWS // 2, j)

            def dma_bt(t):
                bt = padp.tile([128, 10, 82], BF16, tag="bpad")
                nc.sync.dma_start(bt[0:64], bs_d[:, 8 * t:8 * t + 10])
                nc.scalar.dma_start(
                    bt[64:128, 0:9], bs_d[:, 8 * t + 1:8 * t + 10]
                )
                return bt

            bt0 = dma_bt(0)
            for j in range(NCHUNK):
                build_rhs(0, j)
                _copy_chunk(nc, lhsT[0], bt0, 8, j)
                build_rhs(1, j)

            def build_tile(t):
                bt = dma_bt(t)
                for j in range(NCHUNK):
                    _copy_chunk(nc, lhsT[t], bt, 8, j)

            def norm_tile(t):
                # inv_norms for a whole lhsT tile (5 m-tiles, 640 patches)
                # in one batched chain: one bf16 Square on ACT, 4 bf16 DVE
                # adds, 5 small bf16 ones-matmuls (128-partition reduce)
                # into one PSUM tile, one Sqrt/max/reciprocal epilogue.
                sq = sqp.tile([128, NCHUNK, 640], BF16, tag="sq")
                nc.scalar.activation(sq[:], lhsT[t][:], AF.Square)
                # chunk 4's upper half duplicates tap (2,2) (for matmul
                # pairing) -- include it only on partitions 0:64
                ssum = sqp.tile([128, 640], BF16, tag="ssum")
                nc.vector.tensor_add(
                    ssum[0:64, :], sq[0:64, 0, :], sq[0:64, 4, :]
                )
                nc.vector.tensor_copy(ssum[64:128, :], sq[64:128, 0, :])
                nc.vector.tensor_add(ssum[:], ssum[:], sq[:, 1, :])
                nc.vector.tensor_add(ssum[:], ssum[:], sq[:, 2, :])
                ssr = sqp.tile([128, 640], BF16, tag="ssr")
                nc.vector.tensor_add(ssr[:], ssum[:], sq[:, 3, :])
                ps_w = pssp.tile([128, 6], F32, tag="pss")
                for ml in range(5):
                    nc.tensor.matmul(
                        ps_w[:, ml:ml + 2],
                        lhsT=ssr[:, ml * 128:(ml + 1) * 128],
                        rhs=ones[:],
                        start=True, stop=True,
                    )
                inv = invp.tile([128, 5], F32, tag="inv")
                nc.scalar.activation(inv[:], ps_w[:, 0:5], AF.Sqrt)
                nc.vector.tensor_scalar(
                    inv[:], inv[:], EPS, None, mybir.AluOpType.max
                )
                nc.vector.reciprocal(inv[:], inv[:])
                return inv

            def norm_slice(t, ml):
                # single-m-tile norm chain (short critical path): used for
                # t=0 so inv_0 is ready before m=0's scale-copies, keeping
                # the PSUM pool draining from the very start
                msl = slice(ml * 128, (ml + 1) * 128)
                sq = sqp.tile([128, NCHUNK, 128], BF16, tag="sqs")
                nc.scalar.activation(sq[:], lhsT[t][:, :, msl], AF.Square)
                ssum = sqp.tile([128, 128], BF16, tag="ssums")
                nc.vector.tensor_add(
                    ssum[0:64, :], sq[0:64, 0, :], sq[0:64, 4, :]
                )
                nc.vector.tensor_copy(ssum[64:128, :], sq[64:128, 0, :])
                nc.vector.tensor_add(ssum[:], ssum[:], sq[:, 1, :])
                nc.vector.tensor_add(ssum[:], ssum[:], sq[:, 2, :])
                ssr = sqp.tile([128, 128], BF16, tag="ssrs")
                nc.vector.tensor_add(ssr[:], ssum[:], sq[:, 3, :])
                ps_s = pssp.tile([128, 6], F32, tag="pss")
                nc.tensor.matmul(
                    ps_s[:, 0:2], lhsT=ssr[:], rhs=ones[:],
                    start=True, stop=True,
                )
                inv = invp.tile([128, 1], F32, tag="invs")
                nc.scalar.activation(inv[:], ps_s[:, 0:1], AF.Sqrt)
                nc.vector.tensor_scalar(
                    inv[:], inv[:], EPS, None, mybir.AluOpType.max
                )
                nc.vector.reciprocal(inv[:], inv[:])
                return inv

            # norms for the first lhsT tiles issue right after their
            # builds (ahead of the remaining builds), so the m=0..9
            # scale-copies don't stall behind the build queue on ACT
            inv0 = [norm_slice(0, ml) for ml in range(5)]
            build_tile(1)
            inv_of = {1: norm_tile(1)}
            for t in range(2, MT // 5):
                build_tile(t)

            inv_t = None
            for m in range(MT):
                t, ml = divmod(m, 5)
                msl = slice(ml * 128, (ml + 1) * 128)
                tail_dma = m >= MT - 1

                if ml == 0 and t > 0:
                    inv_t = inv_of.get(t)
                    if inv_t is None:
                        inv_t = norm_tile(t)
                inv = inv0[ml] if t == 0 else inv_t[:, ml:ml + 1]
                pstiles = []
                for nt in range(NT):
                    ps = psp.tile([128, NTILE], F32, tag="ps")
                    pstiles.append(ps)
                    for j in range(NCHUNK - 1):
                        nc.tensor.matmul(
                            ps[:],
                            lhsT=lhsT[t][:, j, msl],
                            rhs=rhs[nt // 2][:, j,
                                            (nt % 2) * NTILE:(nt % 2 + 1) * NTILE],
                            start=(j == 0),
                            stop=False,
                        )
                # chunk 4 (tap (2,2), K=64): pair two n-tiles per PE slot
                # via tile_position row groups -- nt 0/2 read the lower
                # half, nt 1/3 the (replica-filled) upper half.  Adjacent
                # disjoint-row-group matmuls run concurrently; trailing
                # the group costs one LDW-transition stall (~90ns) per
                # m-tile, cheaper than any interleaved placement.
                for nt in range(NT):
                    p0 = 64 * (nt % 2)
                    nc.tensor.matmul(
                        pstiles[nt][:],
                        lhsT=lhsT[t][p0:p0 + 64, 4, msl],
                        rhs=rhs[nt // 2][p0:p0 + 64, 4,
                                         (nt % 2) * NTILE:(nt % 2 + 1) * NTILE],
                        start=False,
                        stop=True,
                        tile_position=(p0, 0),
                    )

                # n-tiles in pairs sharing one [128, 800] output staging
                # tile -> one DMA per pair (halves Sync-sequencer issues).
                # The last m-tile instead issues one [128, 400] DMA per
                # n-tile, alternating the two HWDGE rings, so the final
                # transfers start right after each scale-copy.
                for nt0 in range(0, NT, 2):
                    ot = outp.tile([128, 2, NTILE], BF16, tag="ot")
                    for i, nt in enumerate((nt0, nt0 + 1)):
                        # alternate DVE / ACT to balance the two engines
                        if i == 0:
                            nc.vector.tensor_scalar_mul(
                                ot[:, i, :], pstiles[nt][:], inv
                            )
                        else:
                            nc.scalar.activation(
                                ot[:, i, :], pstiles[nt][:], AF.Copy,
                                scale=inv,
                            )
                        if tail_dma:
                            eng = nc.sync if i == 0 else nc.scalar
                            eng.dma_start(
                                y_d[m * 128:(m + 1) * 128,
                                    nt * NTILE:(nt + 1) * NTILE],
                                ot[:, i, :],
                            )
                    if not tail_dma:
                        nc.sync.dma_start(
                            y_d[m * 128:(m + 1) * 128,
                                nt0 * NTILE:(nt0 + 2) * NTILE],
                            ot[:],
                        )
    return nc


def _split_multiwaits(nc, maxw=1):
    """Walrus (this build) accepts at most one sync-wait per instruction.

    Tile's kernel-tail drain carries one wait per active logical proc, so
    hoist excess waits onto same-engine NoOps inserted right before the
    offending instruction (engine executes them in order -> identical
    blocking semantics)."""
    n = 0
    for fn in nc.m.functions:
        for blk in fn.blocks:
            insts = list(blk.instructions)
            new, changed = [], False
            for ins in insts:
                si = ins.sync_info
                if si is not None and len(si.on_wait) > maxw:
                    extra, keep = si.on_wait[:-maxw], si.on_wait[-maxw:]
                    k = 0
                    while extra:
                        chunk, extra = extra[:maxw], extra[maxw:]
                        new.append(mybir.InstNoOp(
                            name=f"{ins.name}-ws{k}",
                            engine=ins.engine,
                            bass_nofuse=True,
                            sync_info=mybir.SyncInfo(
                                on_wait=list(chunk), on_update=[]
                            ),
                        ))
                        k += 1
                        n += 1
                    ins.sync_info = mybir.SyncInfo(
                        on_wait=list(keep), on_update=list(si.on_update)
                    )
                    changed = True
                new.append(ins)
            if changed:
                blk.instructions = new
    return n


_CACHE = {}


def _get_nc():
    if "nc" not in _CACHE:
        nc = build_nc()
        _split_multiwaits(nc)
        _CACHE["nc"] = nc
    return _CACHE["nc"]


def make_in_maps(f, b):
    f = np.asarray(f, dtype=np.float32)
    b = np.asarray(b, dtype=np.float32)
    n_samples = f.shape[0]
    fs = f[:, :, ::2, ::2].astype(NP_BF16)
    bs = b[:, :, ::2, ::2].astype(NP_BF16)
    fpad = np.zeros((n_samples, C, 82, 82), NP_BF16)
    fpad[:, :, 1:81, 1:81] = fs
    bpad = np.zeros((n_samples, C, 82, 82), NP_BF16)
    bpad[:, :, 1:81, 1:81] = bs
    in_maps = []
    for c in range(8):
        n, q = divmod(c, 4)
        in_maps.append({
            "fs_pad": np.ascontiguousarray(fpad[n, :, 20 * q:20 * q + 22, :]),
            "bs_pad": np.ascontiguousarray(bpad[n]),
        })
    return in_maps


def assemble(results, n_samples=2):
    out = np.empty((n_samples, L, H, W), np.float32)
    for c in range(8):
        n, q = divmod(c, 4)
        out[n, :, 20 * q:20 * q + 20, :] = (
            results[c]["y"].astype(np.float32).reshape(L, QROWS, W)
        )
    return out


def run(f, b, **kw):
    res = run_bass_kernel_spmd(_get_nc(), make_in_maps(f, b), list(range(8)), **kw)
    return assemble(res.results, np.asarray(f).shape[0]), res


def kernel(f, b):
    out, _ = run(f, b)
    return out

